# revision 4
# baseline (speedup 1.0000x reference)
"""GAT-with-LSTM-gates kernel for Trainium2, SPMD over 8 NeuronCores. v3

Problem: B=16 graphs, N=1024 nodes, D=128 features.
    h   = x @ Ww.T + Wb
    e   = (h @ A) @ h.T;  e_sym = e + e.T  (== h @ (A + A.T) @ h.T)
    s   = where(adj > 0, e_sym, 0)
    att = softmax(s, axis=1) * adj
    h'  = relu(att @ h)
    ic/fc/oc = sigmoid(h' @ w*_u + x @ w*_x)        (scalar per node)
    out = oc * tanh(ic * h' + fc * x)

Sharding: data-parallel over B; 2 graphs per core; params replicated.

Score decomposition: s[j,k] = x_j M1 x_k + vx_j + vx_k + kap with
M1 = Ww.T As Ww, vx = x @ (Ww.T As Wb), kap = Wb As Wb, As = A + A.T.

Device formulation (per graph), tiles [c=partition (normalization col k),
a=free (softmax axis j)]:
    t[f, a]   = M1 @ x.T                       (PE)
    u[f, a]   = t + v                          (fused into the PSUM->SBUF copy;
                 makes e carry the free-axis bias vx_a so the adjacency needs
                 no prescale and ships as raw 0/1 FP8)
    e[c, a]   = u_strip.T @ x.T = x_a M1 x_c + vx_a     (PE, f32r)
    p         = exp(e)                         (ACT, no bias: the per-partition
                 factor e^{vx_c+kap} cancels between numerator and Z)
    q         = p * adj01                      (Pool; adj01 raw 0/1 in FP8)
    Z[c]      = sum_a q + zinit[c],  zinit = (N-deg_c) e^{-vx_c-kap}
    h_s[c, d] = h_nat * (1/Z)                  (h_nat via DMA X-bar transpose)
    h'T[d, a] = sum_c h_s[c,d] q[c,a]          (PE bf16 accumulation)
    hp        = relu(h'T)                      (DVE max; GPSIMD cannot read
                 PSUM on HW, so all PSUM->SBUF hops live on DVE)
    gates     = per-block PE matmuls hp_blk.T @ U -> gn_ps[128, 24] directly
                 (gate-major cols j*8+b); host (x@wx) added via an
                 I128-identity matmul into the same PSUM accumulation group
    sigmoid   = 0.5 tanh(0.5 z) + 0.5          (tiny ACT from PSUM + DVE)
    out[a, d] = oc * tanh(ic*hp_nat + fc*x_nat)  (bf16; host upcasts)

PSUM plan: ps_a bufs=3 holds only {t, hT, e*8} per graph (keeps the
cross-iteration rotation shallow); ps_hp bufs=1 rotates {hp, gn}.
"""

import contextlib

import numpy as np

import concourse.bacc as bacc
import concourse.bass as bass
import concourse.mybir as mybir
import concourse.tile as tile
from concourse.bass_utils import run_bass_kernel_spmd

F32 = mybir.dt.float32
F32R = mybir.dt.float32r
BF16 = mybir.dt.bfloat16
F8 = mybir.dt.float8e4
AF = mybir.ActivationFunctionType
OP = mybir.AluOpType

B, N, D = 16, 1024, 128
NCORES = 8
GPC = B // NCORES  # graphs per core
NC_TILES = N // 128  # 8 row strips of the [N, N] score matrix

_C_M1 = 0       # [128, 128] Ww.T @ As @ Ww  (score core matrix)
_C_WWT = 128    # [128, 128] Ww.T
_C_COLS = 256
NG = 3 * NC_TILES  # 24 gate columns


def _build_program(reps=1):
    """reps>1 wraps the whole per-call body in a hardware loop — used only
    for benchmarking (amortizes the host->device dispatch overhead)."""
    nc = bacc.Bacc(None, enable_partition_id=False)

    xT = nc.dram_tensor("xT", [GPC, D, N], F32R, kind="ExternalInput")
    xn = nc.dram_tensor("xn", [GPC, 128, NC_TILES * D], BF16,
                        kind="ExternalInput")
    adjT = nc.dram_tensor("adjT", [GPC, N, N], F8, kind="ExternalInput")
    zx = nc.dram_tensor("zx", [GPC, 128, 8], F32, kind="ExternalInput")
    gxb = nc.dram_tensor("gxb", [GPC, 128, NG], BF16, kind="ExternalInput")
    cb_d = nc.dram_tensor("cb", [D, _C_COLS], F32R, kind="ExternalInput")
    cu_d = nc.dram_tensor("cu", [D, 4], BF16, kind="ExternalInput")
    cw_d = nc.dram_tensor("cw", [D, 8], F32, kind="ExternalInput")
    out = nc.dram_tensor("out", [GPC, 128, NC_TILES * D], BF16,
                         kind="ExternalOutput")

    with tile.TileContext(nc) as tc:
        with (
            tc.tile_pool(name="const", bufs=1) as constp,
            tc.tile_pool(name="big", bufs=2) as big,
            tc.tile_pool(name="adjp", bufs=6) as adjp,
            tc.tile_pool(name="qp", bufs=6) as qp,
            tc.tile_pool(name="small", bufs=2) as small,
            tc.tile_pool(name="ps_a", bufs=3, space="PSUM") as ps_a,
            tc.tile_pool(name="ps_hp", bufs=1, space="PSUM") as ps_hp,
        ):
            # ---- constants (loaded once, outside the bench loop) ----
            cb = constp.tile([D, _C_COLS], F32R, name="cb_sb")
            nc.gpsimd.dma_start(out=cb[:, 0:128], in_=cb_d[:, 0:128])
            nc.gpsimd.dma_start(out=cb[:, 128:256], in_=cb_d[:, 128:256])
            cu = constp.tile([D, 4], BF16, name="cu_sb")
            nc.gpsimd.dma_start(out=cu[:], in_=cu_d[:])
            cw = constp.tile([D, 8], F32, name="cw_sb")
            nc.gpsimd.dma_start(out=cw[:], in_=cw_d[:])
            M1 = cb[:, _C_M1:_C_M1 + 128]
            WwT = cb[:, _C_WWT:_C_WWT + 128]
            U = cu[:, 0:3]
            WbC = cw[:, 0:1]
            VC = cw[:, 1:2]

            st = [dict() for _ in range(GPC)]  # per-graph tile state

            def emit_head_dma(g):
                s = st[g]
                xT_sb = big.tile([D, N], F32R, name="xT_sb", tag="xT")
                nc.sync.dma_start(out=xT_sb[:, 0:512], in_=xT[g][:, 0:512])
                nc.sync.dma_start(out=xT_sb[:, 512:1024],
                                  in_=xT[g][:, 512:1024])
                zx_sb = small.tile([128, 8], F32, name="zx_sb", tag="zx")
                nc.sync.dma_start(out=zx_sb[:], in_=zx[g])
                gx_sb = small.tile([128, NG], BF16, name="gx_sb", tag="gx")
                nc.sync.dma_start(out=gx_sb[:], in_=gxb[g])
                s.update(xT=xT_sb, zx=zx_sb, gx=gx_sb, adj=[], e=[])

            def emit_xn(g):
                s = st[g]
                xn_sb = big.tile([128, NC_TILES, D], BF16, name="xn_sb",
                                 tag="xn")
                nc.sync.dma_start(
                    out=xn_sb.rearrange("p a b -> p (a b)"), in_=xn[g])
                s["xn"] = xn_sb

            def emit_head_compute(g):
                s = st[g]
                xT_sb = s["xT"]
                # t = M1 @ x.T; u = t + v fused into the PSUM->SBUF copy
                t_ps = ps_a.tile([D, 2, 512], F32, name="t_ps", tag="A")
                for k in range(2):
                    nc.tensor.matmul(
                        t_ps[:, k, :], M1[:],
                        xT_sb[:, k * 512:(k + 1) * 512],
                        start=True, stop=True)
                # hT = Ww @ x.T (bias added during the DVE copy)
                hT_ps = ps_a.tile([D, 2, 512], F32, name="hT_ps", tag="A")
                for k in range(2):
                    nc.tensor.matmul(
                        hT_ps[:, k, :], WwT[:],
                        xT_sb[:, k * 512:(k + 1) * 512],
                        start=True, stop=True)
                tsb = big.tile([D, N], F32R, name="tsb", tag="tsb")
                for k in range(2):
                    nc.vector.tensor_scalar(
                        tsb[:, k * 512:(k + 1) * 512], t_ps[:, k, :], VC,
                        None, OP.add)
                hTb = big.tile([D, N], BF16, name="hTb", tag="hTb")
                nc.vector.tensor_scalar(
                    hTb[:], hT_ps.rearrange("p a b -> p (a b)"), WbC, None,
                    OP.add)
                # h in natural layout via DMA X-bar transpose
                h_nat = big.tile([128, NC_TILES, D], BF16, name="h_nat",
                                 tag="h_nat")
                nc.sync.dma_start_transpose(out=h_nat[:], in_=hTb[:])
                s.update(tsb=tsb, h_nat=h_nat)
                s["hp_ps"] = None
                s["h_s"] = big.tile([128, NC_TILES, D], BF16, name="h_s",
                                    tag="h_s")

            def emit_adj(g, ci):
                s = st[g]
                adj_sb = adjp.tile([128, N], F8, name="adj_sb", tag="adj")
                nc.sync.dma_start(
                    out=adj_sb[:], in_=adjT[g, ci * 128:(ci + 1) * 128, :])
                s["adj"].append(adj_sb)

            def emit_e(g, ci):
                s = st[g]
                e_ps = ps_a.tile([128, 2, 512], F32, name="e_ps", tag="A")
                for k in range(2):
                    nc.tensor.matmul(
                        e_ps[:, k, :],
                        s["tsb"][:, ci * 128:(ci + 1) * 128],
                        s["xT"][:, k * 512:(k + 1) * 512],
                        start=True, stop=True)
                s["e"].append(e_ps)

            def emit_strip(g, ci):
                s = st[g]
                if s["hp_ps"] is None:
                    s["hp_ps"] = ps_hp.tile([D, 2, 512], F32, name="hp_ps",
                                            tag="hp")
                e_ps = s["e"][ci]
                # p = exp(e) (no shift: |e| <~ 15)
                p_sb = qp.tile([128, N], BF16, name="p_sb", tag="p")
                nc.scalar.activation(
                    p_sb[:], e_ps.rearrange("p a b -> p (a b)"), AF.Exp)
                # mask on Pool (raw 0/1 fp8 adjacency)
                q_sb = qp.tile([128, N], BF16, name="q_sb", tag="q")
                nc.gpsimd.tensor_tensor(
                    q_sb[:], p_sb[:], s["adj"][ci][:], OP.mult)
                # row-sum on DVE (overwrites the dead p tile); Z = sum + zinit
                Zq = small.tile([128, 1], F32, name="Zq", tag="Zq")
                nc.vector.tensor_scalar(
                    p_sb[:], q_sb[:], 1.0, 0.0, OP.mult, OP.add,
                    accum_out=Zq[:])
                Z = small.tile([128, 1], F32, name="Z", tag="Z")
                nc.vector.tensor_scalar(
                    Z[:], Zq[:], s["zx"][:, ci:ci + 1], None, OP.add)
                R = small.tile([128, 1], F32, name="R", tag="R")
                nc.vector.reciprocal(R[:], Z[:])
                nc.vector.tensor_scalar(
                    s["h_s"][:, ci, :], s["h_nat"][:, ci, :], R[:], None,
                    OP.mult)
                # h'T accumulation
                for k in range(2):
                    nc.tensor.matmul(
                        s["hp_ps"][:, k, :],
                        s["h_s"][:, ci, :],
                        q_sb[:, k * 512:(k + 1) * 512],
                        start=(ci == 0), stop=(ci == NC_TILES - 1))

            def emit_tail_a(g):
                """relu (DVE); per-block gate matmuls -> gn_ps [128, 24];
                host (x@wx) accumulated via I128 matmul; hp transpose."""
                s = st[g]
                hp = big.tile([D, N], BF16, name="hp", tag="hp_sb")
                hp_nat = big.tile([128, NC_TILES, D], BF16, name="hp_nat",
                                  tag="hp_nat")
                for k in range(2):
                    nc.vector.tensor_scalar(
                        hp[:, k * 512:(k + 1) * 512], s["hp_ps"][:, k, :],
                        0.0, None, OP.max)
                    nc.sync.dma_start_transpose(
                        out=hp_nat[:, k * 4:(k + 1) * 4, :],
                        in_=hp[:, k * 512:(k + 1) * 512])
                gnp = ps_hp.tile([D, 2, 512], F32, name="gnp", tag="hp")
                gn_ps = gnp[:, 0, 0:NG]
                for ai in range(NC_TILES):
                    # gate j of block ai -> col j*8 + ai
                    nc.tensor.matmul(
                        gn_ps[:, ai:ai + 2 * NC_TILES + 1:NC_TILES],
                        hp[:, ai * 128:(ai + 1) * 128], U,
                        start=True, stop=True)
                s["hp"] = hp
                s["hp_nat"] = hp_nat
                s["gn_ps"] = gn_ps

            def emit_tail_b(g):
                """+ host (x@wx); sigmoid = 0.5 tanh(0.5 z) + 0.5."""
                s = st[g]
                gn_t = small.tile([128, NG], F32, name="gn_t", tag="gn_t")
                nc.vector.tensor_tensor(
                    gn_t[:], s["gn_ps"], s["gx"][:], OP.add)
                gn_h = small.tile([128, NG], F32, name="gn_h", tag="gn_h")
                nc.scalar.activation(gn_h[:], gn_t[:], AF.Tanh,
                                     scale=0.5)
                gn = small.tile([128, NG], BF16, name="gn", tag="gn")
                nc.vector.tensor_scalar(
                    gn[:], gn_h[:], 0.5, 0.5, OP.mult, OP.add)
                s["gn"] = gn

            def emit_tail_cd(g, h):
                """half h: w = ic*h'_nat + fc*x_nat; out = oc*tanh(w)."""
                s = st[g]
                hw = [128, 4, D]
                sl = slice(h * 4, (h + 1) * 4)

                def gbh(j):
                    c0 = j * NC_TILES + h * 4
                    return (s["gn"][:, c0:c0 + 4]
                            .rearrange("p (a u) -> p a u", u=1)
                            .broadcast_to(hw))

                w1 = big.tile(hw, BF16, name="w1", tag=f"w1_{h}")
                v = big.tile(hw, BF16, name="v", tag=f"v_{h}")
                nc.vector.tensor_tensor(w1[:], s["hp_nat"][:, sl, :],
                                        gbh(0), OP.mult)
                nc.gpsimd.tensor_tensor(v[:], s["xn"][:, sl, :], gbh(1),
                                        OP.mult)
                w_all = big.tile(hw, BF16, name="w_all", tag=f"wa_{h}")
                nc.gpsimd.tensor_tensor(w_all[:], w1[:], v[:], OP.add)
                t_all = big.tile(hw, BF16, name="t_all", tag=f"ta_{h}")
                nc.scalar.activation(
                    t_all.rearrange("p a b -> p (a b)"),
                    w_all.rearrange("p a b -> p (a b)"), AF.Tanh)
                out_sb = big.tile(hw, BF16, name="out_sb", tag=f"o_{h}")
                nc.vector.tensor_tensor(out_sb[:], t_all[:], gbh(2),
                                        OP.mult)
                nc.gpsimd.dma_start(
                    out=out[g][:, h * 512:(h + 1) * 512],
                    in_=out_sb.rearrange("p a b -> p (a b)"))

            loop_ctx = (tc.For_i(0, reps, 1) if reps > 1
                        else contextlib.nullcontext())
            with loop_ctx:
                # software-pipelined over the GPC=2 graphs: tail of g
                # overlaps attention of g+1; g+1's head overlaps g's strips
                emit_head_dma(0)
                emit_adj(0, 0)
                emit_adj(0, 1)
                emit_head_compute(0)
                emit_e(0, 0)
                emit_head_dma(1)
                emit_e(0, 1)
                emit_adj(0, 2)
                for ci in range(NC_TILES):
                    if ci + 3 < NC_TILES:
                        emit_adj(0, ci + 3)
                    elif ci + 3 < 2 * NC_TILES - 2:
                        emit_adj(1, ci + 3 - NC_TILES)
                    if ci + 2 < NC_TILES:
                        emit_e(0, ci + 2)
                    emit_strip(0, ci)
                    if ci == 1:
                        emit_head_compute(1)
                    if ci == 2:
                        emit_xn(0)
                emit_e(1, 0)
                emit_e(1, 1)
                emit_e(1, 2)
                emit_tail_a(0)
                emit_tail_b(0)
                emit_xn(1)
                for ci in range(NC_TILES):
                    if ci + 3 < NC_TILES:
                        emit_adj(1, ci + 3)
                    if ci + 3 < NC_TILES:
                        emit_e(1, ci + 3)
                    emit_strip(1, ci)
                    if ci == 1:
                        emit_tail_cd(0, 0)
                    if ci == 3:
                        emit_tail_cd(0, 1)
                emit_tail_a(1)
                emit_tail_b(1)
                emit_tail_cd(1, 0)
                emit_tail_cd(1, 1)
    nc.finalize()
    return nc


_CACHE = {}


def _get_program():
    if "nc" not in _CACHE:
        _CACHE["nc"] = _build_program()
    return _CACHE["nc"]


def _make_consts(inputs):
    bf16 = mybir.dt.np(BF16)
    Ww = np.asarray(inputs["Ww"], np.float64)
    Wb = np.asarray(inputs["Wb"], np.float64)
    A_ = np.asarray(inputs["A"], np.float64)
    As = A_ + A_.T
    v = Ww.T @ As @ Wb
    cb = np.zeros((D, _C_COLS), np.float32)
    cb[:, _C_M1:_C_M1 + 128] = Ww.T @ As @ Ww
    cb[:, _C_WWT:_C_WWT + 128] = Ww.T
    cu = np.zeros((D, 4), np.float32)
    cu[:, 0] = inputs["wi_u"]
    cu[:, 1] = inputs["wf_u"]
    cu[:, 2] = inputs["wo_u"]
    cw = np.zeros((D, 8), np.float32)
    cw[:, 0] = Wb
    cw[:, 1] = v
    return cb, cu.astype(bf16), cw


def prep_inputs(inputs):
    """Host-side layout prep: transposes / packing / dtype casts.
    Returns the in_maps list for run_bass_kernel_spmd."""
    bf16 = mybir.dt.np(BF16)
    f8 = mybir.dt.np(F8)
    x = np.asarray(inputs["x"], np.float32)
    adj = np.asarray(inputs["adj"], np.float32)

    xT_all = np.ascontiguousarray(x.transpose(0, 2, 1))
    xn_all = np.ascontiguousarray(
        x.reshape(B, NC_TILES, 128, D).transpose(0, 2, 1, 3)
        .reshape(B, 128, NC_TILES * D)).astype(bf16)
    adjT_all = np.ascontiguousarray(adj.transpose(0, 2, 1)).astype(f8)

    Ww = np.asarray(inputs["Ww"], np.float64)
    Wb = np.asarray(inputs["Wb"], np.float64)
    A_ = np.asarray(inputs["A"], np.float64)
    As = A_ + A_.T
    v = Ww.T @ As @ Wb
    kap = Wb @ As @ Wb
    vx = (x.astype(np.float64) @ v)  # [B, N]
    # zinit[c] = (N - deg[c]) * e^{-vx_c - kap}: the masked entries'
    # exp(0)=1 contributions to Z, rescaled by the cancelled per-partition
    # softmax factor
    deg = adj.sum(axis=1)  # [B, N]
    zi = ((np.float32(N) - deg) * np.exp(-(vx + kap)))
    zx_all = np.ascontiguousarray(
        zi.reshape(B, NC_TILES, 128).transpose(0, 2, 1)).astype(np.float32)
    # host-folded gate x-projections, gate-major: gxb[p, j*8+b]
    wx = np.stack([inputs["wi_x"], inputs["wf_x"], inputs["wo_x"]],
                  axis=1).astype(np.float64)
    gx = np.einsum("bnf,fi->bni", x.astype(np.float64), wx)  # [B, N, 3]
    gxb_all = np.ascontiguousarray(
        gx.reshape(B, NC_TILES, 128, 3).transpose(0, 2, 3, 1)
        .reshape(B, 128, NG)).astype(bf16)
    cb, cu, cw = _make_consts(inputs)

    in_maps = []
    for c in range(NCORES):
        s = slice(c * GPC, (c + 1) * GPC)
        in_maps.append({
            "xT": xT_all[s],
            "xn": xn_all[s],
            "adjT": adjT_all[s],
            "zx": zx_all[s],
            "gxb": gxb_all[s],
            "cb": cb,
            "cu": cu,
            "cw": cw,
        })
    return in_maps


def unpack_output(res):
    """[NCORES] of out [GPC, 128, NC*D] bf16 -> [B, N, D] fp32."""
    out = np.empty((B, N, D), dtype=np.float32)
    for c in range(NCORES):
        o = np.asarray(res.results[c]["out"], dtype=np.float32)
        out[c * GPC:(c + 1) * GPC] = (
            o.reshape(GPC, 128, NC_TILES, D).transpose(0, 2, 1, 3)
            .reshape(GPC, N, D))
    return out


def kernel(x, adj, Ww, Wb, A, wi_u, wi_x, wf_u, wf_x, wo_u, wo_x):
    inputs = {"x": x, "adj": adj, "Ww": Ww, "Wb": Wb, "A": A,
              "wi_u": wi_u, "wi_x": wi_x, "wf_u": wf_u, "wf_x": wf_x,
              "wo_u": wo_u, "wo_x": wo_x}
    in_maps = prep_inputs(inputs)
    nc = _get_program()
    res = run_bass_kernel_spmd(nc, in_maps, list(range(NCORES)))
    return unpack_output(res)
